# revision 2
# baseline (speedup 1.0000x reference)
"""Data-parallel Trainium kernel for nn_AttnModel_13692355740200.

Shards the batch dimension B=2048 across the 8 NeuronCores (256 targets
per core); the small typed weight tensors are replicated on every core,
per the sharding hint. Each core runs the full per-target-node message
passing (LN -> typed QKV -> relation-prior attention -> typed output
projection -> MergeFFN) on its batch shard; outputs are gathered and
reassembled to full shape on host.
"""

import numpy as np
import jax
import jax.numpy as jnp

B, N, D, DT = 2048, 64, 128, 128
DM = D + DT          # 256
H = 8                # heads, dk = 32
NT, ET = 3, 5        # node / edge type counts
M = 8                # cores
BS = B // M          # 256 per core


def _ln(x, g, b, eps=1e-5):
    mu = jnp.mean(x, -1, keepdims=True)
    var = jnp.mean((x - mu) ** 2, -1, keepdims=True)
    return (x - mu) * jax.lax.rsqrt(var + eps) * g + b


def _shard_fn(src, src_t, seq, seq_t, seq_e, eoh, uoh, mask,
              Wq, Wk, Wv, relation_pri, fc_w, fc_b,
              ln1_g, ln1_b, ln2_g, ln2_b, ln3_g, ln3_b,
              m1_w, m1_b, m2_w, m2_b):
    # eoh: [BS,N,ET+1] one-hot of seq_etype; uoh: [BS,NT+1] one-hot of seq_utype
    q0 = _ln(src, ln1_g, ln1_b)                                # [BS,D]
    qv = jnp.concatenate([q0, src_t[:, 0, :]], -1)             # [BS,DM]
    k0 = _ln(seq + seq_e, ln2_g, ln2_b)                        # [BS,N,D]
    k = jnp.concatenate([k0, seq_t], -1)                       # [BS,N,DM]

    # Q: the query row is identical across N, so project once per edge type
    # and select per (b,n):  qW[b,e] = qv[b] @ Wq[e].T
    qW = jnp.einsum('bi,eoi->beo', qv, Wq)                     # [BS,ET+1,DM]
    Q = jnp.einsum('bne,beo->bno', eoh, qW)                    # [BS,N,DM]

    # K/V: per-row edge-typed projection via mask-and-accumulate
    kf = k.reshape(BS * N, DM)
    K = jnp.zeros((BS * N, DM), jnp.float32)
    V = jnp.zeros((BS * N, DM), jnp.float32)
    for e in range(ET + 1):
        me = eoh[:, :, e].reshape(BS * N, 1)
        ke = kf * me
        K = K + ke @ Wk[e].T
        V = V + ke @ Wv[e].T
    K = K.reshape(BS, N, DM)
    V = V.reshape(BS, N, DM)

    dk = DM // H
    # per-edge, per-head dot product Q.K
    s = jnp.einsum('bnhd,bnhd->hbn',
                   Q.reshape(BS, N, H, dk), K.reshape(BS, N, H, dk))
    pri = jnp.einsum('bu,ue,bne->bn', uoh, relation_pri, eoh)  # [BS,N]
    attn = s * pri[None] / np.sqrt(dk)                         # [H,BS,N]
    attn = jnp.where(mask[None], -1e10, attn)
    attn = jax.nn.softmax(attn, axis=-1)

    Vh = V.reshape(BS, N, H, dk)
    out = jnp.einsum('hbn,bnhd->bhd', attn, Vh).reshape(BS, DM)

    # node-type routed output projection (4 types, masked accumulate)
    fc = jnp.zeros((BS, DM), jnp.float32)
    for u in range(NT + 1):
        fc = fc + (out * uoh[:, u:u + 1]) @ fc_w[u].T
    fc = fc + uoh @ fc_b                                       # [BS,DM]
    fc = _ln(fc, ln3_g, ln3_b)

    hmid = jax.nn.relu(jnp.concatenate([fc, src], 1) @ m1_w.T + m1_b)
    final = hmid @ m2_w.T + m2_b                               # [BS,D]
    return final, attn


_pmapped = None


def _get_pmapped():
    global _pmapped
    if _pmapped is None:
        _pmapped = jax.pmap(
            _shard_fn,
            in_axes=(0, 0, 0, 0, 0, 0, 0, 0) + (None,) * 16,
        )
    return _pmapped


def kernel(src, src_t, seq, seq_t, seq_e, Wq, Wk, Wv, relation_pri,
           fc_w, fc_b, ln1_g, ln1_b, ln2_g, ln2_b, ln3_g, ln3_b,
           m1_w, m1_b, m2_w, m2_b, seq_etype, seq_utype, seq_vtype, mask):
    f32 = np.float32
    # host-side: shard batch tensors [B,...] -> [M,BS,...]; one-hot the
    # integer type tensors (index preprocessing only, no float math)
    sh = lambda a: np.ascontiguousarray(np.asarray(a, f32).reshape((M, BS) + a.shape[1:]))
    et = np.asarray(seq_etype).astype(np.int64)
    ut = np.asarray(seq_utype).astype(np.int64)
    eoh = (et[..., None] == np.arange(ET + 1)).astype(f32).reshape(M, BS, N, ET + 1)
    uoh = (ut[:, None] == np.arange(NT + 1)).astype(f32).reshape(M, BS, NT + 1)
    maskb = np.asarray(mask, bool).reshape(M, BS, N)

    pf = _get_pmapped()
    final, attn = pf(
        sh(src), sh(src_t), sh(seq), sh(seq_t), sh(seq_e), eoh, uoh, maskb,
        np.asarray(Wq, f32), np.asarray(Wk, f32), np.asarray(Wv, f32),
        np.asarray(relation_pri, f32), np.asarray(fc_w, f32),
        np.asarray(fc_b, f32), np.asarray(ln1_g, f32), np.asarray(ln1_b, f32),
        np.asarray(ln2_g, f32), np.asarray(ln2_b, f32), np.asarray(ln3_g, f32),
        np.asarray(ln3_b, f32), np.asarray(m1_w, f32), np.asarray(m1_b, f32),
        np.asarray(m2_w, f32), np.asarray(m2_b, f32),
    )
    final = np.asarray(final).reshape(B, D)
    # attn: [M,H,BS,N] -> [H,B,N] -> [H*B,N]
    attn = np.asarray(attn).transpose(1, 0, 2, 3).reshape(H, B, N).reshape(H * B, N)
    return final.astype(f32), attn.astype(f32)
